# revision 10
# baseline (speedup 1.0000x reference)
"""Trainium2 Bass kernel for BilinearInteraction (v7: 3-lane drain).

Computes out[b,p,:] = (x[:,pi[p],:] @ W[p]) * x[:,pj[p],:] for all P=276
field pairs (pi,pj) = combinations(24, 2), B=2048, E=128.

Strategy (8 NeuronCores):
  - Data-parallel: shard batch (2048 -> 256 rows/core), replicate W.
  - W quantized to fp8 e3m4 on host (rel err ~1.4e-2 < 2e-2 gate);
    bf16 stationary x e3m4 moving matmul verified on HW.
  - Stationary = x_i^T per (group, batch-chunk); moving = contiguous W
    columns; psum [b, p*128+f] tiles of 1536 cols (3 banks).
  - Drain (the bottleneck): every tile's FIRST bank goes directly
    VectorE-fp32-from-PSUM -> store staging, so PSUM frees right after
    the ScalarE copy of banks 1-2.  Copied banks land in a contiguous
    per-supertile (3-tile) arena so the bf16 multiplies run as few,
    wide 3D-AP TTs (pieces of 1024 at stride 1536).  Two supertiles
    per chunk are multiplied on GpSimd instead of VectorE (3rd lane).
  - DMA on BOTH HWDGE rings: stores + 2 head loads on SP ring; all
    remaining loads queued up-front on the Scalar (ACT) ring.
  - Per-tile HAM warm matmuls read the previous tile's staging slice
    (data dependency) so the tile scheduler cannot hoist them into one
    PE-blocking run.
"""

import numpy as np
import ml_dtypes

# ---------------------------------------------------------------- constants
F = 24          # fields
E = 128         # embedding dim
B = 2048        # batch
P = F * (F - 1) // 2        # 276 pairs
NCORES = 8
B_LOCAL = B // NCORES       # 256 rows per core
BCH = 2                     # batch chunks of 128
COLS = P * E                # 35328 output columns per batch chunk
HALF = F * E                # 3072: per-chunk x columns

PAIRS = [(i, j) for i in range(F) for j in range(i + 1, F)]  # p -> (i,j)
GS = [F - 1 - g for g in range(F - 1)]                       # group sizes
GP = [0]
for s in GS:
    GP.append(GP[-1] + s)                                    # pair start per group

BANK = 512                  # fp32 elems per PSUM bank
TCOLS = 3 * BANK            # 1536 cols per psum tile (3 banks)
NT = COLS // TCOLS          # 23 tiles per batch chunk
DCOLS = BANK                # direct (VectorE-from-PSUM) cols per tile
CCOLS = TCOLS - DCOLS       # copied cols per tile (1024)
ST = 3                      # tiles per supertile / store chunk
NS = (NT + ST - 1) // ST    # 8 supertiles (last has 2 tiles)
GPS_SUPER = (2, 5)          # supertiles whose bf16 muls run on GpSimd
WARM_PRE = 10               # dummy matmuls before the first real one
WARM_PER_TILE = 1           # dummy matmuls appended per tile (HAM boost)

W_CHUNKS = [(0, 32), (32, 128), (128, 224), (224, 276)]


def _group_of_col(x):
    p = x // E
    for g in range(len(GS)):
        if GP[g] <= p < GP[g + 1]:
            return g
    raise AssertionError


def _segs(lo, hi):
    """Matmul segs for cols [lo,hi): cut at PSUM banks and group edges.

    Returns (lo, hi, g, start, stop); start/stop are per-bank flags.
    """
    cuts = {lo, hi}
    c = (lo // BANK) * BANK
    while c < hi:
        if lo < c < hi:
            cuts.add(c)
        c += BANK
    for g in range(1, len(GS)):
        e = GP[g] * E
        if lo < e < hi:
            cuts.add(e)
    cuts = sorted(cuts)
    segs = []
    for k in range(len(cuts) - 1):
        a, b = cuts[k], cuts[k + 1]
        assert b - a <= BANK
        segs.append([a, b, _group_of_col(a)])
    out = []
    for k, (a, b, g) in enumerate(segs):
        bank = a // BANK
        first = k == 0 or segs[k - 1][0] // BANK != bank
        last = k == len(segs) - 1 or segs[k + 1][0] // BANK != bank
        out.append((a, b, g, first, last))
    return out


def _moff(col):
    """Multiplier column for output col: j(col//128)*128 + col%128."""
    return PAIRS[col // E][1] * E + col % E


def _runs(lo, hi):
    """Split cols [lo,hi) into maximal runs with a contiguous multiplier
    (cut at group edges).  Returns (lo, hi, mult_off) triples."""
    out = []
    c = lo
    while c < hi:
        g = _group_of_col(c)
        e = min(hi, GP[g + 1] * E)
        out.append((c, e, _moff(c)))
        c = e
    return out


def _merged_runs(pieces):
    """pieces: list of (global_lo, global_hi) copied regions (width CCOLS,
    global stride TCOLS).  Returns
      - merged: (k0, n, moff0) 3D runs over n consecutive FULL pieces in
        one multiplier-contiguous group,
      - singles: (global_lo, global_hi, moff) 1D runs for the rest.
    """
    full = []
    for k, (lo, hi) in enumerate(pieces):
        if _group_of_col(lo) == _group_of_col(hi - 1):
            full.append(k)
    merged, singles = [], []
    run = []
    def flush_run():
        if len(run) >= 2:
            lo = pieces[run[0]][0]
            merged.append((run[0], len(run), _moff(lo)))
        else:
            for k in run:
                lo, hi = pieces[k]
                singles.append((lo, hi, _moff(lo)))
        run.clear()
    for k, (lo, hi) in enumerate(pieces):
        ok = k in full
        if ok and run and run[-1] == k - 1:
            # contiguous multiplier across pieces requires same group
            if _group_of_col(pieces[run[0]][0]) == _group_of_col(lo):
                run.append(k)
                continue
        flush_run()
        if ok:
            run.append(k)
        else:
            singles.extend(_runs(lo, hi))
    flush_run()
    return merged, singles


def _build_schedule():
    supers = []
    for s in range(NS):
        tlo = s * ST
        thi = min(NT, tlo + ST)
        tiles = []
        pieces = []
        for t in range(tlo, thi):
            t0 = t * TCOLS
            tiles.append(dict(
                t0=t0,
                segs=_segs(t0, t0 + TCOLS),
                direct_runs=_runs(t0, t0 + DCOLS),
            ))
            pieces.append((t0 + DCOLS, t0 + TCOLS))
        merged, singles = _merged_runs(pieces)
        supers.append(dict(
            s=s,
            s0=tlo * TCOLS,
            n_so=(thi - tlo) * TCOLS,
            n_pieces=thi - tlo,
            tiles=tiles,
            merged=merged,
            singles=singles,
            gps=s in GPS_SUPER,
        ))
    return supers


def _dedup_ldweights(nc, mybir):
    """Remove InstLdweights whose weights AP matches the previous LDW on
    the PE stream (the stationary is still loaded); move any syncs onto
    the following instruction.  Verified correct on HW (probe_dedup)."""
    removed = 0
    for f in nc.m.functions:
        for bb in f.blocks:
            insts = bb.instructions
            last_key = None
            to_remove = []
            for idx, inst in enumerate(insts):
                tn = type(inst).__name__
                if tn == 'InstLdweights':
                    key = str(inst.ins[0])
                    if key == last_key:
                        to_remove.append((idx, inst))
                    last_key = key
                elif tn in ('InstMatmult', 'InstMatmultMx', 'InstEventSemaphore',
                            'InstDrain'):
                    pass
                elif getattr(inst, 'engine', None) == mybir.EngineType.PE:
                    last_key = None
            for idx, inst in reversed(to_remove):
                si = inst.sync_info
                if si is not None and (si.on_wait or si.on_update):
                    nxt = insts[idx + 1]
                    nsi = nxt.sync_info
                    if nsi is None:
                        nxt.sync_info = si
                    else:
                        nsi.on_wait.extend(si.on_wait)
                        nsi.on_update.extend(si.on_update)
                        nxt.sync_info = nsi
                insts.remove(inst)
                removed += 1
    return removed


SUPERS = _build_schedule()

_NC = None


def _build_module():
    global _NC
    if _NC is not None:
        return _NC

    import concourse.bass as bass
    import concourse.tile as tile
    from concourse import bacc, mybir

    bf = mybir.dt.bfloat16
    f8 = mybir.dt.float8e3
    f32 = mybir.dt.float32

    nc = bacc.Bacc("TRN2", target_bir_lowering=False, debug=False)

    # xT[e, c*3072 + f*128 + b]; xn[b, c*3072 + f*128 + e]
    xT = nc.declare_dram_parameter("xT", [E, BCH * HALF], bf, isOutput=False)
    xn = nc.declare_dram_parameter("xn", [E, BCH * HALF], bf, isOutput=False)
    Wt = nc.declare_dram_parameter("Wt", [E, COLS], f8, isOutput=False)
    out = nc.declare_dram_parameter("out", [B_LOCAL, COLS], bf, isOutput=True)

    def pieces3(ap_tile, base, n, width, stride):
        """3D view: n pieces of `width` cols at `stride`, from col `base`."""
        v = ap_tile[:, base: base + n * stride]
        r = v.rearrange("p (a r) -> p a r", r=stride)
        return r[:, :, 0:width]

    with tile.TileContext(nc) as tc:
        with (
            tc.tile_pool(name="const", bufs=1) as cpool,
            tc.tile_pool(name="mm", bufs=4) as mmpool,
            tc.tile_pool(name="so", bufs=8) as sopool,
            tc.tile_pool(name="ps", bufs=2, space=bass.MemorySpace.PSUM) as pspool,
            tc.tile_pool(name="tr", bufs=1, space=bass.MemorySpace.PSUM) as trpool,
        ):
            # xn gets TCOLS of slack so 3D piece views never overflow.
            xT_sb = cpool.tile([E, BCH * HALF], bf, tag="xT")
            xn_sb = cpool.tile([E, BCH * HALF + TCOLS], bf, tag="xn")
            w_sb = [cpool.tile([E, (phi - plo) * E], f8, tag=f"w{k}",
                               name=f"w{k}")
                    for k, (plo, phi) in enumerate(W_CHUNKS)]

            def load_w(eng, k):
                plo, phi = W_CHUNKS[k]
                eng.dma_start(out=w_sb[k][:], in_=Wt[:, plo * E: phi * E])

            # ---- head loads on the SP ring (gate the first tile).
            XA = 4 * E          # first xT piece: fields 0-3
            nc.sync.dma_start(out=xT_sb[:, 0:XA], in_=xT[:, 0:XA])
            load_w(nc.sync, 0)

            # ---- remaining loads queued up-front on the ACT ring.  xn0
            # first: it gates the first tensor_muls.
            nc.scalar.dma_start(out=xn_sb[:, 0:HALF], in_=xn[:, 0:HALF])
            nc.scalar.dma_start(out=xT_sb[:, XA:HALF], in_=xT[:, XA:HALF])
            for k in range(1, len(W_CHUNKS)):
                load_w(nc.scalar, k)
            nc.scalar.dma_start(out=xT_sb[:, HALF:], in_=xT[:, HALF:])
            nc.scalar.dma_start(out=xn_sb[:, HALF:BCH * HALF],
                                in_=xn[:, HALF:])

            # PE warm-up (HAM clock boost).  Per-tile warms depend on the
            # previous tile's staging slice so they execute interleaved.
            warm_sb = cpool.tile([E, BANK], bf, tag="warm")
            trash_ps = trpool.tile([E, BANK], f32, tag="trash")
            nc.vector.memzero(warm_sb[:])

            def warm(n, dep=None):
                mov = warm_sb[:] if dep is None else dep
                for _ in range(n):
                    nc.tensor.matmul(trash_ps[:, 0:mov.shape[-1]],
                                     warm_sb[:, 0:E],
                                     mov, start=True, stop=True)

            warm(WARM_PRE)

            pending = []

            def flush_pending():
                while pending:
                    pending.pop(0)()

            def wchunk_of_pair(p):
                for k, (plo, phi) in enumerate(W_CHUNKS):
                    if plo <= p < phi:
                        return k
                raise AssertionError

            warm_dep = None
            for c in range(BCH):
                for sd in SUPERS:
                    s0 = sd["s0"]
                    n_so = sd["n_so"]
                    so_t = sopool.tile([E, n_so + DCOLS], bf, tag="so")
                    arena = mmpool.tile([E, sd["n_pieces"] * CCOLS], bf,
                                        tag="mm")
                    for k, ti in enumerate(sd["tiles"]):
                        t0 = ti["t0"]
                        ps = pspool.tile([E, TCOLS], f32, tag="ps")
                        for (a, b, g, first, last) in ti["segs"]:
                            ck = wchunk_of_pair(a // E)
                            wlo = a - W_CHUNKS[ck][0] * E
                            nc.tensor.matmul(
                                ps[:, a - t0: b - t0],
                                xT_sb[:, c * HALF + g * E:
                                      c * HALF + (g + 1) * E],
                                w_sb[ck][:, wlo: wlo + (b - a)],
                                start=first, stop=last,
                            )
                        nc.scalar.copy(
                            out=arena[:, k * CCOLS: (k + 1) * CCOLS],
                            in_=ps[:, DCOLS:TCOLS],
                        )
                        for (a, b, moff) in ti["direct_runs"]:
                            nc.vector.tensor_mul(
                                so_t[:, a - s0: b - s0],
                                ps[:, a - t0: b - t0],
                                xn_sb[:, c * HALF + moff:
                                      c * HALF + moff + (b - a)],
                            )
                        flush_pending()
                        warm(WARM_PER_TILE, dep=warm_dep)
                        warm_dep = so_t[:, k * TCOLS: k * TCOLS + BANK]

                    def deferred(c=c, sd=sd, so_t=so_t, arena=arena,
                                 s0=s0, n_so=n_so):
                        mul = (nc.gpsimd.tensor_mul if sd["gps"]
                               else nc.vector.tensor_mul)
                        for (k0, n, moff0) in sd["merged"]:
                            dst = pieces3(so_t, k0 * TCOLS + DCOLS, n,
                                          CCOLS, TCOLS)
                            src = pieces3(arena, k0 * CCOLS, n,
                                          CCOLS, CCOLS)
                            mlt = pieces3(xn_sb, c * HALF + moff0, n,
                                          CCOLS, TCOLS)
                            mul(dst, src, mlt)
                        for (a, b, moff) in sd["singles"]:
                            # arena col of global col a
                            ka = (a - s0) // TCOLS
                            alo = ka * CCOLS + (a - s0 - ka * TCOLS - DCOLS)
                            mul(
                                so_t[:, a - s0: b - s0],
                                arena[:, alo: alo + (b - a)],
                                xn_sb[:, c * HALF + moff:
                                      c * HALF + moff + (b - a)],
                            )
                        nc.sync.dma_start(
                            out=out[c * E: (c + 1) * E, s0: s0 + n_so],
                            in_=so_t[:, 0:n_so],
                        )

                    pending.append(deferred)
            flush_pending()

    _dedup_ldweights(nc, mybir)
    nc.compile()
    _NC = nc
    return nc


def _prep_inputs(x, W):
    """Host-side shard + relayout + quantize. Returns in_maps for 8 cores."""
    bf = ml_dtypes.bfloat16
    e3 = ml_dtypes.float8_e3m4
    x = np.ascontiguousarray(x, dtype=np.float32)
    W = np.ascontiguousarray(W, dtype=np.float32)

    s_w = 15.0 / float(np.abs(W).max())

    # Wt[e, p*128+f] = W[p,e,f] * s_w   (e3m4)
    Wt = np.ascontiguousarray(
        (W * s_w).transpose(1, 0, 2).reshape(E, COLS)
    ).astype(e3)

    in_maps = []
    for core in range(NCORES):
        xs = x[core * B_LOCAL: (core + 1) * B_LOCAL]      # [256, 24, 128]
        xc = xs.reshape(BCH, E, F, E)                     # [c, b, f, e]
        # xT[e, c*3072 + f*128 + b]
        xTh = np.ascontiguousarray(
            xc.transpose(3, 0, 2, 1).reshape(E, BCH * HALF)
        ).astype(bf)
        # xn[b, c*3072 + f*128 + e], pre-divided by s_w
        xnh = np.ascontiguousarray(
            (xc / s_w).transpose(1, 0, 2, 3).reshape(E, BCH * HALF)
        ).astype(bf)
        in_maps.append({"xT": xTh, "xn": xnh, "Wt": Wt})
    return in_maps


def run_on_hw(x, W, trace=False, **run_kwargs):
    """Run the kernel on the 8 NeuronCores; returns (output fp32, results)."""
    from concourse.bass_utils import run_bass_kernel_spmd

    nc = _build_module()
    in_maps = _prep_inputs(x, W)
    res = run_bass_kernel_spmd(
        nc, in_maps, list(range(NCORES)), trace=trace, **run_kwargs
    )
    shards = []
    for core in range(NCORES):
        o = np.asarray(res.results[core]["out"]).astype(np.float32)
        shards.append(o.reshape(B_LOCAL, P, E))
    return np.ascontiguousarray(np.concatenate(shards, axis=0)), res


def kernel(x, W):
    import os
    try:
        out, _ = run_on_hw(x, W, trace=False)
    except Exception:
        # transient device wedge: retry once with a core reset
        os.environ["NEURON_RT_RESET_CORES"] = "1"
        out, _ = run_on_hw(x, W, trace=False)
    return out
